# revision 20
# baseline (speedup 1.0000x reference)
"""Multi-head attention (B=4, S=2048, E=1024, H=16, D=64) on 8 TRN2 NeuronCores.

Sharding: tensor-parallel over heads -- core c computes heads 2c and 2c+1.
Each core receives the full x (cast bf16 and pre-transposed on the host to
[B, 8, 128, S] so every device DMA is a plain linear load) plus its
[E, 128] slices of Wq/Wk/Wv, and produces yT_aug[b, h, 65, S]; the host
normalizes (y / denom), transposes back to [B, S, 128c:128c+128], adds bv,
and concatenates along the feature dim.  Host prep/post is not part of the
device-timed region.

Per-core dataflow (all layouts chosen so no operand ever needs a transpose
at matmul time):
  xT [E-chunk=128, S] (bf16)  -- linear DMA from the host-transposed x
  qT = (Wq^T xT)/8 + bq/8   [128(d,2 heads), S]   (PE + DVE psum->sbuf)
  kT =  Wk^T xT + bk        [128, S]
  vT =  Wv^T xT             [128, S] --PE transpose--> v [S, 128] (+ ones col)
  scoresT[sk, sq] = kT^T qT (K=64 per head; both heads packed in one
        [128,1024] PSUM tile) --ACT Exp(x + maskbias)--> expT bf16
  yT_aug[65, sq] += v_aug^T expT   (K=128; row 'ones' gives softmax denom)
  yT_aug --DVE copy--> SBUF --DMA--> HBM   (normalize + transpose on host)

The emission order software-pipelines batches: batch b+1's projection
matmuls are interleaved into batch b's (ACT-bound) attention loop so the
TensorEngine never waits on the softmax Exp.
"""

import os
import sys
import types

import numpy as np
import ml_dtypes

import concourse.bass as bass
import concourse.tile as tile
from concourse import bacc, mybir
from concourse.bass_utils import run_bass_kernel_spmd
from concourse.masks import make_identity

B, S, E, H, D = 4, 2048, 1024, 16, 64
NCORES = 8
DHC = (H // NCORES) * D  # 128 feature cols per core (2 heads)
NEG = -1.0e9  # additive mask bias for masked-out keys
BF16 = mybir.dt.bfloat16
F32 = mybir.dt.float32
SK = S // 128  # 16 key tiles per batch
SQ = S // 512  # 4 query blocks per batch

LAST_RESULTS = None  # BassKernelResults of the most recent kernel() call


def _install_trace_hook():
    """Register the axon NTFF-profile hook so BASS_TRACE=1 works.

    The concourse trace path imports antenv.axon_hooks, which this image
    doesn't ship; synthesize it and register the ctypes-based hook.
    """
    try:
        import antenv

        if "antenv.axon_hooks" in sys.modules:
            return
        mod = types.ModuleType("antenv.axon_hooks")
        _hook = [None]
        mod.set_axon_ntff_profile_hook = lambda h: _hook.__setitem__(0, h)
        mod.get_axon_ntff_profile_hook = lambda: _hook[0]
        sys.modules["antenv.axon_hooks"] = mod
        antenv.axon_hooks = mod
        from trn_agent_boot.trn_boot import _ntff_profile_via_ctypes

        so = "/opt/axon/libaxon_pjrt.so"
        if os.path.exists(so):
            mod.set_axon_ntff_profile_hook(_ntff_profile_via_ctypes(so))
    except Exception:
        pass


_install_trace_hook()


class _Ctx:
    """Shared emission state for one core's program."""


def _setup(nc, tc, ctx, aps):
    s = _Ctx()
    (s.x, wq, bq, s.out) = aps

    singles = ctx.enter_context(tc.tile_pool(name="singles", bufs=1))
    s.xt_pool = ctx.enter_context(tc.tile_pool(name="xt", bufs=16))
    s.qk_pool = ctx.enter_context(tc.tile_pool(name="qk", bufs=4))
    s.v_pool = ctx.enter_context(tc.tile_pool(name="v", bufs=2))
    s.vt_pool = ctx.enter_context(tc.tile_pool(name="vt", bufs=2))
    s.exp_pool = ctx.enter_context(tc.tile_pool(name="exp", bufs=8))
    s.store_pool = ctx.enter_context(tc.tile_pool(name="store", bufs=4))
    # PSUM budget (8 banks): scores 2x[128,1024]=4, PV accum 3x[128,512]=3
    # (3-deep so the cross-block PV carry never waits on the tail copies),
    # projection accum + v transposes 1x[128,512]=1.
    s.ps_pool = ctx.enter_context(tc.tile_pool(name="ps", bufs=2, space="PSUM"))
    s.py_pool = ctx.enter_context(tc.tile_pool(name="py", bufs=3, space="PSUM"))
    s.prj_pool = ctx.enter_context(tc.tile_pool(name="prj", bufs=1, space="PSUM"))

    # wq gets its own DMA so the first projection matmul only waits on it;
    # wk/wv follow on the same HWDGE queue.
    wcat_sb = singles.tile([128, 3, 8, 128], BF16, tag="wcat")
    nc.scalar.dma_start(out=wcat_sb[:, 0, :, :], in_=wq[:, 0])
    nc.scalar.dma_start(out=wcat_sb[:, 1:3, :, :], in_=wq[:, 1:3])
    s.w_sb = {"wq": wcat_sb[:, 0], "wk": wcat_sb[:, 1], "wv": wcat_sb[:, 2]}
    consts_sb = singles.tile([128, 66], F32, tag="consts")
    nc.scalar.dma_start(out=consts_sb[:, :], in_=bq)
    s.bq_sb = consts_sb[:, 0:1]
    s.bk_sb = consts_sb[:, 1:2]
    s.maskb = consts_sb  # bias for (b, i) at column 2 + 16*b + i
    s.ident_bf = singles.tile([128, 128], BF16, tag="ident_bf")
    make_identity(nc, s.ident_bf[:, :])
    return s


def _gen_proj(nc, s, b, split=False):
    """Generator: emits batch b's xT loads + q/k/v projections.

    Registers output tiles in s.proj[b] up front. Emits [q block 0, all k,
    all v], yields "KV" (attention(b) may start: it needs all of kT/v but
    only qT block 0), then the remaining q blocks -- those are consumed by
    attention(b) only from its second j-block on, so they can spill into
    attention(b)'s early i-steps and fill the batch-boundary bubble.
    """
    mult, add = mybir.AluOpType.mult, mybir.AluOpType.add

    qT = s.qk_pool.tile([128, S], BF16, tag="qk", name=f"qT{b}")
    kT = s.qk_pool.tile([128, S], BF16, tag="qk", name=f"kT{b}")
    v_sb = s.v_pool.tile([128, SK, 192], BF16, tag="v", name=f"v{b}")
    s.proj = getattr(s, "proj", {})
    s.proj[b] = (qT, kT, v_sb)

    xt = []
    for c in range(8):
        t = s.xt_pool.tile([128, S], BF16, tag="xt", name=f"xt{b}_{c}")
        xt.append(t)
    if split:
        # quarters, in the order the projection groups consume them, so the
        # first group can start after a quarter of the batch-0 load
        for q in range(4):
            for c in range(8):
                nc.sync.dma_start(
                    out=xt[c][:, 512 * q:512 * (q + 1)],
                    in_=s.x[b, c, :, 512 * q:512 * (q + 1)])
    else:
        for c in range(8):
            nc.sync.dma_start(out=xt[c][:, :], in_=s.x[b, c, :, :])
    nc.vector.memset(v_sb[:, :, 129:192], 0.0)
    nc.vector.memset(v_sb[:, :, 64:65], 1.0)
    yield "c"

    def q_or_k(name, dest, bias_sb, scale, jhs):
        w = s.w_sb[name]
        for jh in jhs:
            ps = s.prj_pool.tile([128, 512], F32, tag="prj", name="pj")
            for c in range(8):
                nc.tensor.matmul(
                    ps[:, :], w[:, c, :], xt[c][:, 512 * jh:512 * (jh + 1)],
                    start=(c == 0), stop=(c == 7))
                if c % 2 == 1:
                    yield "c"
            nc.vector.tensor_scalar(
                out=dest[:, 512 * jh:512 * (jh + 1)], in0=ps[:, :],
                scalar1=scale, scalar2=bias_sb[:, :], op0=mult, op1=add)
            yield "c"

    def v_proj(jh):
        # v: project to vT, then PE-transpose back to natural [s, d] layout
        # with a fused ones-column (denominator) and 128-wide pad (FWL).
        w = s.w_sb["wv"]
        ps = s.prj_pool.tile([128, 512], F32, tag="prj", name="pv")
        for c in range(8):
            nc.tensor.matmul(
                ps[:, :], w[:, c, :], xt[c][:, 512 * jh:512 * (jh + 1)],
                start=(c == 0), stop=(c == 7))
            if c % 2 == 1:
                yield "c"
        vt = s.vt_pool.tile([128, 512], BF16, tag="vt", name="vt")
        nc.vector.tensor_copy(out=vt[:, :], in_=ps[:, :])
        yield "c"
        for t in range(4):
            i = 4 * jh + t
            pv = s.prj_pool.tile([128, 128], BF16, tag="prj", name="pvt")
            nc.tensor.transpose(
                pv[:, :], vt[:, 128 * t:128 * (t + 1)], s.ident_bf[:, :])
            nc.vector.tensor_copy(out=v_sb[:, i, 0:64], in_=pv[:, 0:64])
            nc.vector.tensor_copy(out=v_sb[:, i, 65:129],
                                  in_=pv[:, 64:128])
            yield "c"

    yield from q_or_k("wq", qT, s.bq_sb, 0.125, [0])
    for jh in range(4):
        yield from q_or_k("wk", kT, s.bk_sb, 1.0, [jh])
        yield from v_proj(jh)
    yield "KV"
    yield from q_or_k("wq", qT, s.bq_sb, 0.125, [1, 2, 3])


def _gen_attn_all(nc, s):
    """Generator: emits all batches' attention; yields per sk tile.

    Each (b, j) block's last 3 PV matmuls (the software-pipeline drain) are
    carried into the NEXT block's first 3 i-steps, where the PE is
    otherwise light (the new block's own PV hasn't filled its lag yet) --
    so neither engine sees a bubble at block or batch boundaries.  Yields
    ("batch", b) before a batch's first scores so the caller can gate on
    proj(b) and queue proj(b+1); yields ("step",) once per i-step.
    """
    pending_tail = None
    carry = None  # (py, v_sb, exps, b, j) of the previous block
    DONE = object()

    for b in range(B):
        yield ("batch", b)
        qT, kT, v_sb = s.proj[b]
        for j in range(SQ):
            jsl = slice(512 * j, 512 * (j + 1))
            py = [s.py_pool.tile([128, 512], F32, tag="py", name=f"py{h}")
                  for h in range(2)]
            exps = []
            for i in range(SK):
                ps = s.ps_pool.tile([128, 1024], F32, tag="ps", name="psc")
                for h in range(2):
                    hp = slice(64 * h, 64 * (h + 1))
                    nc.tensor.matmul(
                        ps[:, 512 * h:512 * (h + 1)],
                        kT[hp, 128 * i:128 * (i + 1)], qT[hp, jsl],
                        start=True, stop=True)
                ex = s.exp_pool.tile([128, 1024], BF16, tag="exp", name="ex")
                nc.scalar.activation(
                    out=ex[:, :], in_=ps[:, :],
                    func=mybir.ActivationFunctionType.Exp,
                    bias=s.maskb[:, 2 + 16 * b + i:3 + 16 * b + i], scale=1.0)
                exps.append(ex)
                # software-pipelined PV: lag three tiles behind scores/exp;
                # steps 0-2 run the previous block's drain instead
                if i > 2:
                    _pv_mms(nc, py, v_sb, exps[i - 3], i - 3)
                elif carry is not None:
                    cpy, cv, cexps, cb, cj = carry
                    _pv_mms(nc, cpy, cv, cexps[SK - 3 + i], SK - 3 + i)
                    if i == 2:
                        while pending_tail is not None:
                            if next(pending_tail, DONE) is DONE:
                                pending_tail = None
                        pending_tail = _gen_tail(nc, s, cb, cj, cpy)
                        carry = None
                if pending_tail is not None and next(pending_tail, DONE) is DONE:
                    pending_tail = None
                yield ("step",)
            carry = (py, v_sb, exps, b, j)

    cpy, cv, cexps, cb, cj = carry
    for ii in (SK - 3, SK - 2, SK - 1):
        _pv_mms(nc, cpy, cv, cexps[ii], ii)
    while pending_tail is not None:
        if next(pending_tail, DONE) is DONE:
            pending_tail = None
    for _ in _gen_tail(nc, s, cb, cj, cpy, last=True):
        pass


def _gen_tail(nc, s, b, j, py, last=False):
    """Store one (b, j) block's raw yT_aug (y rows + denom row) as bf16.

    Normalization (y/denom), the transpose back to [s, d], and the bv add
    all happen on the host after the gather.  The final block splits into
    quarters so the copy->store chain drains fast at the kernel tail.
    """
    nq = 4 if last else 1
    w = 512 // nq
    sts = []
    for h in range(2):
        st = s.store_pool.tile([128, 512], BF16, tag="store", name=f"st{h}")
        sts.append(st)
    for q in range(nq):
        qsl = slice(w * q, w * (q + 1))
        for h in range(2):
            nc.vector.tensor_copy(out=sts[h][0:65, qsl], in_=py[h][0:65, qsl])
            nc.sync.dma_start(
                out=s.out[b, h, :, 512 * j + w * q:512 * j + w * (q + 1)],
                in_=sts[h][0:65, qsl])
        if not last:
            yield


def _pv_mms(nc, py, v_sb, ex, i):
    for h in range(2):
        nc.tensor.matmul(
            py[h][:, :], v_sb[:, i, 64 * h:64 * h + 128],
            ex[:, 512 * h:512 * (h + 1)],
            start=(i == 0), stop=(i == SK - 1))


def _emit_body(nc, tc, ctx, aps):
    from collections import deque

    s = _setup(nc, tc, ctx, aps)
    # Tile derives dependencies from emission order, so every projection
    # block must be emitted before the attention matmul that reads it.
    # Projection chunks (~2 matmuls each) are interleaved into the previous
    # batch's attention emission so the TensorEngine stays fed while the
    # (serial) softmax Exp chain runs.  Before attention(b) starts, proj(b)
    # must have reached its "KV" marker (all of kT/v plus qT block 0); the
    # trailing q blocks spill into attention(b)'s early i-steps, which are
    # otherwise PE-light (the PV software pipeline hasn't filled yet).
    pending = deque()
    kv_seen = {}
    gp = {b: _gen_proj(nc, s, b, split=(b == 0)) for b in range(B)}

    def drain_one():
        if not pending:
            return
        tok = next(pending[0], None)
        if tok is None:
            pending.popleft()
        elif tok == "KV":
            kv_seen[pending[0]] = True

    pending.append(gp[0])
    for tok in _gen_attn_all(nc, s):
        if tok[0] == "batch":
            b = tok[1]
            # proj(b) must be emitted through its KV marker (all kT/v plus
            # qT block 0) before this batch's first scores matmul
            while pending and not kv_seen.get(gp[b]):
                drain_one()
            if b + 1 < B:
                pending.append(gp[b + 1])
        else:
            # batch-0 leftovers at 4x so attention(0) never outruns the q
            # blocks it reads (emission order defines deps)
            n = 4 if (pending and pending[0] is gp[0]) else 1
            for _ in range(n):
                drain_one()
    while pending:
        drain_one()


def _build():
    from contextlib import ExitStack

    nc = bacc.Bacc("TRN2", target_bir_lowering=False, debug=False)
    x = nc.dram_tensor("x", [B, 8, 128, S], BF16, kind="ExternalInput").ap()
    wq = nc.dram_tensor("wcat", [128, 3, 8, 128], BF16,
                        kind="ExternalInput").ap()
    bq = nc.dram_tensor("consts", [128, 66], F32, kind="ExternalInput").ap()
    out = nc.dram_tensor("out", [B, 2, 65, S], BF16, kind="ExternalOutput").ap()
    aps = (x, wq, bq, out)
    with tile.TileContext(nc) as tc:
        with ExitStack() as ctx:
            _emit_body(nc, tc, ctx, aps)
    nc.compile()
    return nc


_BUILD_CACHE = {}


def _get_built():
    if "nc" not in _BUILD_CACHE:
        _BUILD_CACHE["nc"] = _build()
    return _BUILD_CACHE["nc"]


def kernel(x, mask, Wq, bq, Wk, bk, Wv, bv):
    global LAST_RESULTS
    bf16 = ml_dtypes.bfloat16
    x_bf = np.asarray(x, dtype=np.float32).astype(bf16)
    # [B, S, E] -> [B, 8, 128, S]: E-chunk-transposed on the host so every
    # device-side DMA is a plain linear load.
    x_t = np.ascontiguousarray(
        x_bf.reshape(B, S, 8, 128).transpose(0, 2, 3, 1))
    mask_f = np.asarray(mask).astype(np.float32)
    maskb = (mask_f - 1.0) * (-NEG)  # 0 where mask==1, NEG where mask==0
    maskb = np.ascontiguousarray(
        maskb.reshape(B, S // 128, 128).transpose(2, 0, 1)).astype(np.float32)

    nc = _get_built()

    in_maps = []
    for c in range(NCORES):
        sl = slice(DHC * c, DHC * (c + 1))

        def warr(w):
            w = np.asarray(w, dtype=np.float32)[:, sl].astype(bf16)
            return np.ascontiguousarray(
                w.reshape(8, 128, 128).transpose(1, 0, 2))

        wcat = np.stack([warr(Wq), warr(Wk), warr(Wv)], axis=1)
        consts = np.empty((128, 66), dtype=np.float32)
        consts[:, 0] = np.asarray(bq, dtype=np.float32)[sl] / 8.0
        consts[:, 1] = np.asarray(bk, dtype=np.float32)[sl]
        consts[:, 2:66] = maskb.reshape(128, 64)
        in_maps.append({
            "x": x_t,
            "wcat": np.ascontiguousarray(wcat),
            "consts": consts,
        })

    res = run_bass_kernel_spmd(nc, in_maps, core_ids=list(range(NCORES)))
    LAST_RESULTS = res

    # Host-side unshard: normalize by the softmax denominator row, put the
    # heads back on the feature axis, and add bv (exact: softmax weights
    # sum to 1, so y = sum(w * (v + bv)) == sum(w * v) + bv).
    parts = []
    for c in range(NCORES):
        r = np.asarray(res.results[c]["out"], dtype=np.float32)  # [B,2,65,S]
        # v_aug packing: head0 rows 0:64 are y with denom in row 64;
        # head1 row 0 is denom with y in rows 1:65.
        y0 = r[:, 0, 0:64, :] / r[:, 0, 64:65, :]
        y1 = r[:, 1, 1:65, :] / r[:, 1, 0:1, :]
        y = np.stack([y0, y1], axis=1)  # [B, 2, 64, S]
        parts.append(y.transpose(0, 3, 1, 2).reshape(B, S, DHC))
    full = np.concatenate(parts, axis=-1)
    bvf = np.asarray(bv, dtype=np.float32)
    if np.any(bvf != 0):
        full = full + bvf[None, None, :]
    return np.ascontiguousarray(full, dtype=np.float32)



# revision 23
# speedup vs baseline: 1.0156x; 1.0156x over previous
"""Multi-head attention (B=4, S=2048, E=1024, H=16, D=64) on 8 TRN2 NeuronCores.

Sharding: tensor-parallel over heads -- core c computes heads 2c and 2c+1.
Each core receives the full x (cast bf16 and pre-transposed on the host to
[B, 8, 128, S] so every device DMA is a plain linear load) plus its
[E, 128] slices of Wq/Wk/Wv, and produces yT_aug[b, h, 65, S]; the host
normalizes (y / denom), transposes back to [B, S, 128c:128c+128], adds bv,
and concatenates along the feature dim.  Host prep/post is not part of the
device-timed region.

Per-core dataflow (all layouts chosen so no operand ever needs a transpose
at matmul time):
  xT [E-chunk=128, S] (bf16)  -- linear DMA from the host-transposed x
  qT = (Wq^T xT)/8 + bq/8   [128(d,2 heads), S]   (PE + DVE psum->sbuf)
  kT =  Wk^T xT + bk        [128, S]
  vT =  Wv^T xT             [128, S] --PE transpose--> v [S, 128] (+ ones col)
  scoresT[sk, sq] = kT^T qT (K=64 per head; both heads packed in one
        [128,1024] PSUM tile) --ACT Exp(x + maskbias)--> expT bf16
  yT_aug[65, sq] += v_aug^T expT   (K=128; row 'ones' gives softmax denom)
  yT_aug --DVE copy--> SBUF --DMA--> HBM   (normalize + transpose on host)

The emission order software-pipelines batches: batch b+1's projection
matmuls are interleaved into batch b's (ACT-bound) attention loop so the
TensorEngine never waits on the softmax Exp.
"""

import os
import sys
import types

import numpy as np
import ml_dtypes

import concourse.bass as bass
import concourse.tile as tile
from concourse import bacc, mybir
from concourse.bass_utils import run_bass_kernel_spmd
from concourse.masks import make_identity

B, S, E, H, D = 4, 2048, 1024, 16, 64
NCORES = 8
DHC = (H // NCORES) * D  # 128 feature cols per core (2 heads)
NEG = -1.0e9  # additive mask bias for masked-out keys
BF16 = mybir.dt.bfloat16
F32 = mybir.dt.float32
SK = S // 128  # 16 key tiles per batch
SQ = S // 512  # 4 query blocks per batch

LAST_RESULTS = None  # BassKernelResults of the most recent kernel() call


def _install_trace_hook():
    """Register the axon NTFF-profile hook so BASS_TRACE=1 works.

    The concourse trace path imports antenv.axon_hooks, which this image
    doesn't ship; synthesize it and register the ctypes-based hook.
    """
    try:
        import antenv

        if "antenv.axon_hooks" in sys.modules:
            return
        mod = types.ModuleType("antenv.axon_hooks")
        _hook = [None]
        mod.set_axon_ntff_profile_hook = lambda h: _hook.__setitem__(0, h)
        mod.get_axon_ntff_profile_hook = lambda: _hook[0]
        sys.modules["antenv.axon_hooks"] = mod
        antenv.axon_hooks = mod
        from trn_agent_boot.trn_boot import _ntff_profile_via_ctypes

        so = "/opt/axon/libaxon_pjrt.so"
        if os.path.exists(so):
            mod.set_axon_ntff_profile_hook(_ntff_profile_via_ctypes(so))
    except Exception:
        pass


_install_trace_hook()


class _Ctx:
    """Shared emission state for one core's program."""


def _setup(nc, tc, ctx, aps):
    s = _Ctx()
    (s.x, wq, bq, s.out) = aps

    singles = ctx.enter_context(tc.tile_pool(name="singles", bufs=1))
    s.xt_pool = ctx.enter_context(tc.tile_pool(name="xt", bufs=16))
    s.qk_pool = ctx.enter_context(tc.tile_pool(name="qk", bufs=4))
    s.v_pool = ctx.enter_context(tc.tile_pool(name="v", bufs=2))
    s.vt_pool = ctx.enter_context(tc.tile_pool(name="vt", bufs=2))
    s.exp_pool = ctx.enter_context(tc.tile_pool(name="exp", bufs=8))
    s.store_pool = ctx.enter_context(tc.tile_pool(name="store", bufs=4))
    # PSUM budget (8 banks): scores 2x[128,1024]=4, PV accum 2x[128,512]=2,
    # projection accum + v transposes 2x[128,512]=2.
    s.ps_pool = ctx.enter_context(tc.tile_pool(name="ps", bufs=2, space="PSUM"))
    s.py_pool = ctx.enter_context(tc.tile_pool(name="py", bufs=2, space="PSUM"))
    s.prj_pool = ctx.enter_context(tc.tile_pool(name="prj", bufs=2, space="PSUM"))

    # wq gets its own DMA, first on the sync ring (which starts earliest),
    # so the first projection matmul only waits on it; wk/wv go on the
    # scalar ring in parallel with the xt loads.
    wcat_sb = singles.tile([128, 3, 8, 128], BF16, tag="wcat")
    nc.sync.dma_start(out=wcat_sb[:, 0, :, :], in_=wq[:, 0])
    nc.scalar.dma_start(out=wcat_sb[:, 1:3, :, :], in_=wq[:, 1:3])
    s.w_sb = {"wq": wcat_sb[:, 0], "wk": wcat_sb[:, 1], "wv": wcat_sb[:, 2]}
    consts_sb = singles.tile([128, 66], F32, tag="consts")
    nc.scalar.dma_start(out=consts_sb[:, :], in_=bq)
    s.bq_sb = consts_sb[:, 0:1]
    s.bk_sb = consts_sb[:, 1:2]
    s.maskb = consts_sb  # bias for (b, i) at column 2 + 16*b + i
    s.ident_bf = singles.tile([128, 128], BF16, tag="ident_bf")
    make_identity(nc, s.ident_bf[:, :])
    return s


def _gen_proj(nc, s, b, split=False):
    """Generator: emits batch b's xT loads + q/k/v projections.

    Registers output tiles in s.proj[b] up front. Emits [q block 0, all k,
    all v], yields "KV" (attention(b) may start: it needs all of kT/v but
    only qT block 0), then the remaining q blocks -- those are consumed by
    attention(b) only from its second j-block on, so they can spill into
    attention(b)'s early i-steps and fill the batch-boundary bubble.
    """
    mult, add = mybir.AluOpType.mult, mybir.AluOpType.add

    qT = s.qk_pool.tile([128, S], BF16, tag="qk", name=f"qT{b}")
    kT = s.qk_pool.tile([128, S], BF16, tag="qk", name=f"kT{b}")
    v_sb = s.v_pool.tile([128, SK, 192], BF16, tag="v", name=f"v{b}")
    s.proj = getattr(s, "proj", {})
    s.proj[b] = (qT, kT, v_sb)

    xt = []
    for c in range(8):
        t = s.xt_pool.tile([128, S], BF16, tag="xt", name=f"xt{b}_{c}")
        xt.append(t)
    if split:
        # quarters, in the order the projection groups consume them, so the
        # first group can start after a quarter of the batch-0 load
        for q in range(4):
            for c in range(8):
                nc.sync.dma_start(
                    out=xt[c][:, 512 * q:512 * (q + 1)],
                    in_=s.x[b, c, :, 512 * q:512 * (q + 1)])
    else:
        for c in range(8):
            nc.sync.dma_start(out=xt[c][:, :], in_=s.x[b, c, :, :])
    nc.vector.memset(v_sb[:, :, 129:192], 0.0)
    nc.vector.memset(v_sb[:, :, 64:65], 1.0)
    yield "c"

    def q_or_k(name, dest, bias_sb, scale, jhs):
        w = s.w_sb[name]
        for jh in jhs:
            ps = s.prj_pool.tile([128, 512], F32, tag="prj", name="pj")
            for c in range(8):
                nc.tensor.matmul(
                    ps[:, :], w[:, c, :], xt[c][:, 512 * jh:512 * (jh + 1)],
                    start=(c == 0), stop=(c == 7))
                if c % 2 == 1:
                    yield "c"
            nc.vector.tensor_scalar(
                out=dest[:, 512 * jh:512 * (jh + 1)], in0=ps[:, :],
                scalar1=scale, scalar2=bias_sb[:, :], op0=mult, op1=add)
            yield "c"

    def v_proj(jh):
        # v: project to vT, then PE-transpose back to natural [s, d] layout
        # with a fused ones-column (denominator) and 128-wide pad (FWL).
        w = s.w_sb["wv"]
        ps = s.prj_pool.tile([128, 512], F32, tag="prj", name="pv")
        for c in range(8):
            nc.tensor.matmul(
                ps[:, :], w[:, c, :], xt[c][:, 512 * jh:512 * (jh + 1)],
                start=(c == 0), stop=(c == 7))
            if c % 2 == 1:
                yield "c"
        vt = s.vt_pool.tile([128, 512], BF16, tag="vt", name="vt")
        nc.vector.tensor_copy(out=vt[:, :], in_=ps[:, :])
        yield "c"
        for t in range(4):
            i = 4 * jh + t
            pv = s.prj_pool.tile([128, 128], BF16, tag="prj", name="pvt")
            nc.tensor.transpose(
                pv[:, :], vt[:, 128 * t:128 * (t + 1)], s.ident_bf[:, :])
            nc.vector.tensor_copy(out=v_sb[:, i, 0:64], in_=pv[:, 0:64])
            nc.vector.tensor_copy(out=v_sb[:, i, 65:129],
                                  in_=pv[:, 64:128])
            yield "c"

    yield from q_or_k("wq", qT, s.bq_sb, 0.125, [0])
    for jh in range(4):
        yield from q_or_k("wk", kT, s.bk_sb, 1.0, [jh])
        yield from v_proj(jh)
    yield "KV"
    yield from q_or_k("wq", qT, s.bq_sb, 0.125, [1, 2, 3])


def _gen_attn_all(nc, s):
    """Generator: emits all batches' attention; yields per sk tile.

    Each j-block's store tail is deferred and dripped into the next
    j-block's main loop so the tail never stalls the scores->exp->PV
    pipeline at j boundaries.  Yields ("batch", b) before a batch's first
    scores so the caller can gate on proj(b) and queue proj(b+1); yields
    ("step",) once per i-step.
    """
    pending_tail = None
    DONE = object()

    for b in range(B):
        yield ("batch", b)
        qT, kT, v_sb = s.proj[b]
        for j in range(SQ):
            jsl = slice(512 * j, 512 * (j + 1))
            py = [s.py_pool.tile([128, 512], F32, tag="py", name=f"py{h}")
                  for h in range(2)]
            exps = []
            for i in range(SK):
                ps = s.ps_pool.tile([128, 1024], F32, tag="ps", name="psc")
                for h in range(2):
                    hp = slice(64 * h, 64 * (h + 1))
                    nc.tensor.matmul(
                        ps[:, 512 * h:512 * (h + 1)],
                        kT[hp, 128 * i:128 * (i + 1)], qT[hp, jsl],
                        start=True, stop=True)
                ex = s.exp_pool.tile([128, 1024], BF16, tag="exp", name="ex")
                nc.scalar.activation(
                    out=ex[:, :], in_=ps[:, :],
                    func=mybir.ActivationFunctionType.Exp,
                    bias=s.maskb[:, 2 + 16 * b + i:3 + 16 * b + i], scale=1.0)
                exps.append(ex)
                # software-pipelined PV: lag three tiles behind scores/exp
                if i > 2:
                    _pv_mms(nc, py, v_sb, exps[i - 3], i - 3)
                if pending_tail is not None and next(pending_tail, DONE) is DONE:
                    pending_tail = None
                yield ("step",)
            for ii in (SK - 3, SK - 2, SK - 1):
                _pv_mms(nc, py, v_sb, exps[ii], ii)
            while pending_tail is not None:  # should be drained already
                if next(pending_tail, DONE) is DONE:
                    pending_tail = None
            last = (b == B - 1 and j == SQ - 1)
            pending_tail = _gen_tail(nc, s, b, j, py, last=last)
    while pending_tail is not None:
        if next(pending_tail, DONE) is DONE:
            pending_tail = None


def _gen_tail(nc, s, b, j, py, last=False):
    """Store one (b, j) block's raw yT_aug (y rows + denom row) as bf16.

    Normalization (y/denom), the transpose back to [s, d], and the bv add
    all happen on the host after the gather.  The final block splits into
    quarters so the copy->store chain drains fast at the kernel tail.
    """
    nq = 4 if last else 1
    w = 512 // nq
    sts = []
    for h in range(2):
        st = s.store_pool.tile([128, 512], BF16, tag="store", name=f"st{h}")
        sts.append(st)
    for q in range(nq):
        qsl = slice(w * q, w * (q + 1))
        for h in range(2):
            nc.vector.tensor_copy(out=sts[h][0:65, qsl], in_=py[h][0:65, qsl])
            nc.sync.dma_start(
                out=s.out[b, h, :, 512 * j + w * q:512 * j + w * (q + 1)],
                in_=sts[h][0:65, qsl])
        if not last:
            yield


def _pv_mms(nc, py, v_sb, ex, i):
    for h in range(2):
        nc.tensor.matmul(
            py[h][:, :], v_sb[:, i, 64 * h:64 * h + 128],
            ex[:, 512 * h:512 * (h + 1)],
            start=(i == 0), stop=(i == SK - 1))


def _emit_body(nc, tc, ctx, aps):
    from collections import deque

    s = _setup(nc, tc, ctx, aps)
    # Tile derives dependencies from emission order, so every projection
    # block must be emitted before the attention matmul that reads it.
    # Projection chunks (~2 matmuls each) are interleaved into the previous
    # batch's attention emission so the TensorEngine stays fed while the
    # (serial) softmax Exp chain runs.  Before attention(b) starts, proj(b)
    # must have reached its "KV" marker (all of kT/v plus qT block 0); the
    # trailing q blocks spill into attention(b)'s early i-steps, which are
    # otherwise PE-light (the PV software pipeline hasn't filled yet).
    pending = deque()
    kv_seen = {}
    gp = {b: _gen_proj(nc, s, b, split=(b == 0)) for b in range(B)}

    def drain_one():
        if not pending:
            return
        tok = next(pending[0], None)
        if tok is None:
            pending.popleft()
        elif tok == "KV":
            kv_seen[pending[0]] = True

    pending.append(gp[0])
    for tok in _gen_attn_all(nc, s):
        if tok[0] == "batch":
            b = tok[1]
            # proj(b) must be emitted through its KV marker (all kT/v plus
            # qT block 0) before this batch's first scores matmul
            while pending and not kv_seen.get(gp[b]):
                drain_one()
            if b + 1 < B:
                pending.append(gp[b + 1])
        else:
            # batch-0 leftovers at 4x so attention(0) never outruns the q
            # blocks it reads (emission order defines deps)
            n = 4 if (pending and pending[0] is gp[0]) else 1
            for _ in range(n):
                drain_one()
    while pending:
        drain_one()


def _build():
    from contextlib import ExitStack

    nc = bacc.Bacc("TRN2", target_bir_lowering=False, debug=False)
    x = nc.dram_tensor("x", [B, 8, 128, S], BF16, kind="ExternalInput").ap()
    wq = nc.dram_tensor("wcat", [128, 3, 8, 128], BF16,
                        kind="ExternalInput").ap()
    bq = nc.dram_tensor("consts", [128, 66], F32, kind="ExternalInput").ap()
    out = nc.dram_tensor("out", [B, 2, 65, S], BF16, kind="ExternalOutput").ap()
    aps = (x, wq, bq, out)
    with tile.TileContext(nc) as tc:
        with ExitStack() as ctx:
            _emit_body(nc, tc, ctx, aps)
    nc.compile()
    return nc


_BUILD_CACHE = {}


def _get_built():
    if "nc" not in _BUILD_CACHE:
        _BUILD_CACHE["nc"] = _build()
    return _BUILD_CACHE["nc"]


def kernel(x, mask, Wq, bq, Wk, bk, Wv, bv):
    global LAST_RESULTS
    bf16 = ml_dtypes.bfloat16
    x_bf = np.asarray(x, dtype=np.float32).astype(bf16)
    # [B, S, E] -> [B, 8, 128, S]: E-chunk-transposed on the host so every
    # device-side DMA is a plain linear load.
    x_t = np.ascontiguousarray(
        x_bf.reshape(B, S, 8, 128).transpose(0, 2, 3, 1))
    mask_f = np.asarray(mask).astype(np.float32)
    maskb = (mask_f - 1.0) * (-NEG)  # 0 where mask==1, NEG where mask==0
    maskb = np.ascontiguousarray(
        maskb.reshape(B, S // 128, 128).transpose(2, 0, 1)).astype(np.float32)

    nc = _get_built()

    in_maps = []
    for c in range(NCORES):
        sl = slice(DHC * c, DHC * (c + 1))

        def warr(w):
            w = np.asarray(w, dtype=np.float32)[:, sl].astype(bf16)
            return np.ascontiguousarray(
                w.reshape(8, 128, 128).transpose(1, 0, 2))

        wcat = np.stack([warr(Wq), warr(Wk), warr(Wv)], axis=1)
        consts = np.empty((128, 66), dtype=np.float32)
        consts[:, 0] = np.asarray(bq, dtype=np.float32)[sl] / 8.0
        consts[:, 1] = np.asarray(bk, dtype=np.float32)[sl]
        consts[:, 2:66] = maskb.reshape(128, 64)
        in_maps.append({
            "x": x_t,
            "wcat": np.ascontiguousarray(wcat),
            "consts": consts,
        })

    res = run_bass_kernel_spmd(nc, in_maps, core_ids=list(range(NCORES)))
    LAST_RESULTS = res

    # Host-side unshard: normalize by the softmax denominator row, put the
    # heads back on the feature axis, and add bv (exact: softmax weights
    # sum to 1, so y = sum(w * (v + bv)) == sum(w * v) + bv).
    parts = []
    for c in range(NCORES):
        r = np.asarray(res.results[c]["out"], dtype=np.float32)  # [B,2,65,S]
        # v_aug packing: head0 rows 0:64 are y with denom in row 64;
        # head1 row 0 is denom with y in rows 1:65.
        y0 = r[:, 0, 0:64, :] / r[:, 0, 64:65, :]
        y1 = r[:, 1, 1:65, :] / r[:, 1, 0:1, :]
        y = np.stack([y0, y1], axis=1)  # [B, 2, 64, S]
        parts.append(y.transpose(0, 3, 1, 2).reshape(B, S, DHC))
    full = np.concatenate(parts, axis=-1)
    bvf = np.asarray(bv, dtype=np.float32)
    if np.any(bvf != 0):
        full = full + bvf[None, None, :]
    return np.ascontiguousarray(full, dtype=np.float32)



# revision 25
# speedup vs baseline: 1.0326x; 1.0167x over previous
"""Multi-head attention (B=4, S=2048, E=1024, H=16, D=64) on 8 TRN2 NeuronCores.

Sharding: tensor-parallel over heads -- core c computes heads 2c and 2c+1.
Each core receives the full x (cast bf16 and pre-transposed on the host to
[B, 8, 128, S] so every device DMA is a plain linear load) plus its
[E, 128] slices of Wq/Wk/Wv, and produces yT_aug[b, h, 65, S]; the host
normalizes (y / denom), transposes back to [B, S, 128c:128c+128], adds bv,
and concatenates along the feature dim.  Host prep/post is not part of the
device-timed region.

Per-core dataflow (all layouts chosen so no operand ever needs a transpose
at matmul time):
  xT [E-chunk=128, S] (bf16)  -- linear DMA from the host-transposed x
  qT = (Wq^T xT)/8 + bq/8   [128(d,2 heads), S]   (PE + DVE psum->sbuf)
  kT =  Wk^T xT + bk        [128, S]
  vT =  Wv^T xT             [128, S] --PE transpose--> v [S, 128] (+ ones col)
  scoresT[sk, sq] = kT^T qT (K=64 per head; both heads packed in one
        [128,1024] PSUM tile) --ACT Exp(x + maskbias)--> expT bf16
  yT_aug[65, sq] += v_aug^T expT   (K=128; row 'ones' gives softmax denom)
  yT_aug --DVE copy--> SBUF --DMA--> HBM   (normalize + transpose on host)

The emission order software-pipelines batches: batch b+1's projection
matmuls are interleaved into batch b's (ACT-bound) attention loop so the
TensorEngine never waits on the softmax Exp.
"""

import os
import sys
import types

import numpy as np
import ml_dtypes

import concourse.bass as bass
import concourse.tile as tile
from concourse import bacc, mybir
from concourse.bass_utils import run_bass_kernel_spmd
from concourse.masks import make_identity

B, S, E, H, D = 4, 2048, 1024, 16, 64
NCORES = 8
DHC = (H // NCORES) * D  # 128 feature cols per core (2 heads)
NEG = -1.0e9  # additive mask bias for masked-out keys
BF16 = mybir.dt.bfloat16
F32 = mybir.dt.float32
SK = S // 128  # 16 key tiles per batch
SQ = S // 512  # 4 query blocks per batch

LAST_RESULTS = None  # BassKernelResults of the most recent kernel() call


def _install_trace_hook():
    """Register the axon NTFF-profile hook so BASS_TRACE=1 works.

    The concourse trace path imports antenv.axon_hooks, which this image
    doesn't ship; synthesize it and register the ctypes-based hook.
    """
    try:
        import antenv

        if "antenv.axon_hooks" in sys.modules:
            return
        mod = types.ModuleType("antenv.axon_hooks")
        _hook = [None]
        mod.set_axon_ntff_profile_hook = lambda h: _hook.__setitem__(0, h)
        mod.get_axon_ntff_profile_hook = lambda: _hook[0]
        sys.modules["antenv.axon_hooks"] = mod
        antenv.axon_hooks = mod
        from trn_agent_boot.trn_boot import _ntff_profile_via_ctypes

        so = "/opt/axon/libaxon_pjrt.so"
        if os.path.exists(so):
            mod.set_axon_ntff_profile_hook(_ntff_profile_via_ctypes(so))
    except Exception:
        pass


_install_trace_hook()


class _Ctx:
    """Shared emission state for one core's program."""


def _setup(nc, tc, ctx, aps):
    s = _Ctx()
    (s.x, wq, bq, s.out) = aps

    singles = ctx.enter_context(tc.tile_pool(name="singles", bufs=1))
    s.xt_pool = ctx.enter_context(tc.tile_pool(name="xt", bufs=16))
    s.qk_pool = ctx.enter_context(tc.tile_pool(name="qk", bufs=4))
    s.v_pool = ctx.enter_context(tc.tile_pool(name="v", bufs=2))
    s.vt_pool = ctx.enter_context(tc.tile_pool(name="vt", bufs=2))
    s.exp_pool = ctx.enter_context(tc.tile_pool(name="exp", bufs=8))
    s.store_pool = ctx.enter_context(tc.tile_pool(name="store", bufs=4))
    # PSUM budget (8 banks): scores 2x[128,1024]=4, PV accum 2x[128,512]=2,
    # projection accum + v transposes 2x[128,512]=2.
    s.ps_pool = ctx.enter_context(tc.tile_pool(name="ps", bufs=2, space="PSUM"))
    s.py_pool = ctx.enter_context(tc.tile_pool(name="py", bufs=2, space="PSUM"))
    s.prj_pool = ctx.enter_context(tc.tile_pool(name="prj", bufs=2, space="PSUM"))

    # wq gets its own DMA so the first projection matmul only waits on it;
    # wk/wv follow on the same HWDGE queue.
    wcat_sb = singles.tile([128, 3, 8, 128], BF16, tag="wcat")
    nc.scalar.dma_start(out=wcat_sb[:, 0, :, :], in_=wq[:, 0])
    nc.scalar.dma_start(out=wcat_sb[:, 1:3, :, :], in_=wq[:, 1:3])
    s.w_sb = {"wq": wcat_sb[:, 0], "wk": wcat_sb[:, 1], "wv": wcat_sb[:, 2]}
    consts_sb = singles.tile([128, 66], F32, tag="consts")
    nc.scalar.dma_start(out=consts_sb[:, :], in_=bq)
    s.bq_sb = consts_sb[:, 0:1]
    s.bk_sb = consts_sb[:, 1:2]
    s.maskb = consts_sb  # bias for (b, i) at column 2 + 16*b + i
    s.ident_bf = singles.tile([128, 128], BF16, tag="ident_bf")
    make_identity(nc, s.ident_bf[:, :])
    return s


def _gen_proj(nc, s, b, split=False):
    """Generator: emits batch b's xT loads + q/k/v projections.

    Registers output tiles in s.proj[b] up front. Emits [q block 0, all k,
    all v], yields "KV" (attention(b) may start: it needs all of kT/v but
    only qT block 0), then the remaining q blocks -- those are consumed by
    attention(b) only from its second j-block on, so they can spill into
    attention(b)'s early i-steps and fill the batch-boundary bubble.
    """
    mult, add = mybir.AluOpType.mult, mybir.AluOpType.add

    qT = s.qk_pool.tile([128, S], BF16, tag="qk", name=f"qT{b}")
    kT = s.qk_pool.tile([128, S], BF16, tag="qk", name=f"kT{b}")
    v_sb = s.v_pool.tile([128, SK, 192], BF16, tag="v", name=f"v{b}")
    s.proj = getattr(s, "proj", {})
    s.proj[b] = (qT, kT, v_sb)

    xt = []
    for c in range(8):
        t = s.xt_pool.tile([128, S], BF16, tag="xt", name=f"xt{b}_{c}")
        xt.append(t)
    if split:
        # quarters, in the order the projection groups consume them, so the
        # first group can start after a quarter of the batch-0 load
        for q in range(4):
            for c in range(8):
                nc.sync.dma_start(
                    out=xt[c][:, 512 * q:512 * (q + 1)],
                    in_=s.x[b, c, :, 512 * q:512 * (q + 1)])
    else:
        for c in range(8):
            nc.sync.dma_start(out=xt[c][:, :], in_=s.x[b, c, :, :])
    nc.vector.memset(v_sb[:, :, 129:192], 0.0)
    nc.vector.memset(v_sb[:, :, 64:65], 1.0)
    yield "c"

    def q_or_k(name, dest, bias_sb, scale, jhs):
        w = s.w_sb[name]
        for jh in jhs:
            ps = s.prj_pool.tile([128, 512], F32, tag="prj", name="pj")
            for c in range(8):
                nc.tensor.matmul(
                    ps[:, :], w[:, c, :], xt[c][:, 512 * jh:512 * (jh + 1)],
                    start=(c == 0), stop=(c == 7))
                if c % 2 == 1:
                    yield "c"
            nc.vector.tensor_scalar(
                out=dest[:, 512 * jh:512 * (jh + 1)], in0=ps[:, :],
                scalar1=scale, scalar2=bias_sb[:, :], op0=mult, op1=add)
            yield "c"

    def v_proj(jh):
        # v: project to vT, then PE-transpose back to natural [s, d] layout
        # with a fused ones-column (denominator) and 128-wide pad (FWL).
        w = s.w_sb["wv"]
        ps = s.prj_pool.tile([128, 512], F32, tag="prj", name="pv")
        for c in range(8):
            nc.tensor.matmul(
                ps[:, :], w[:, c, :], xt[c][:, 512 * jh:512 * (jh + 1)],
                start=(c == 0), stop=(c == 7))
            if c % 2 == 1:
                yield "c"
        vt = s.vt_pool.tile([128, 512], BF16, tag="vt", name="vt")
        nc.vector.tensor_copy(out=vt[:, :], in_=ps[:, :])
        yield "c"
        for t in range(4):
            i = 4 * jh + t
            pv = s.prj_pool.tile([128, 128], BF16, tag="prj", name="pvt")
            nc.tensor.transpose(
                pv[:, :], vt[:, 128 * t:128 * (t + 1)], s.ident_bf[:, :])
            nc.vector.tensor_copy(out=v_sb[:, i, 0:64], in_=pv[:, 0:64])
            nc.vector.tensor_copy(out=v_sb[:, i, 65:129],
                                  in_=pv[:, 64:128])
            yield "c"

    yield from q_or_k("wq", qT, s.bq_sb, 0.125, [0])
    for jh in range(4):
        yield from q_or_k("wk", kT, s.bk_sb, 1.0, [jh])
        yield from v_proj(jh)
    yield "KV"
    yield from q_or_k("wq", qT, s.bq_sb, 0.125, [1, 2, 3])


def _gen_attn_all(nc, s):
    """Generator: emits all batches' attention; yields per sk tile.

    Each j-block's store tail is deferred and dripped into the next
    j-block's main loop so the tail never stalls the scores->exp->PV
    pipeline at j boundaries.  Yields ("batch", b) before a batch's first
    scores so the caller can gate on proj(b) and queue proj(b+1); yields
    ("step",) once per i-step.
    """
    pending_tail = None
    DONE = object()

    for b in range(B):
        yield ("batch", b)
        qT, kT, v_sb = s.proj[b]
        for j in range(SQ):
            jsl = slice(512 * j, 512 * (j + 1))
            py = [s.py_pool.tile([128, 512], F32, tag="py", name=f"py{h}")
                  for h in range(2)]
            exps = []
            for i in range(SK):
                ps = s.ps_pool.tile([128, 1024], F32, tag="ps", name="psc")
                for h in range(2):
                    hp = slice(64 * h, 64 * (h + 1))
                    nc.tensor.matmul(
                        ps[:, 512 * h:512 * (h + 1)],
                        kT[hp, 128 * i:128 * (i + 1)], qT[hp, jsl],
                        start=True, stop=True)
                ex = s.exp_pool.tile([128, 1024], BF16, tag="exp", name="ex")
                nc.scalar.activation(
                    out=ex[:, :], in_=ps[:, :],
                    func=mybir.ActivationFunctionType.Exp,
                    bias=s.maskb[:, 2 + 16 * b + i:3 + 16 * b + i], scale=1.0)
                exps.append(ex)
                # software-pipelined PV: lag three tiles behind scores/exp
                if i > 2:
                    _pv_mms(nc, py, v_sb, exps[i - 3], i - 3)
                if pending_tail is not None and next(pending_tail, DONE) is DONE:
                    pending_tail = None
                yield ("step",)
            for ii in (SK - 3, SK - 2, SK - 1):
                _pv_mms(nc, py, v_sb, exps[ii], ii)
            while pending_tail is not None:  # should be drained already
                if next(pending_tail, DONE) is DONE:
                    pending_tail = None
            last = (b == B - 1 and j == SQ - 1)
            pending_tail = _gen_tail(nc, s, b, j, py, last=last)
    while pending_tail is not None:
        if next(pending_tail, DONE) is DONE:
            pending_tail = None


def _gen_tail(nc, s, b, j, py, last=False):
    """Store one (b, j) block's raw yT_aug (y rows + denom row) as bf16.

    Normalization (y/denom), the transpose back to [s, d], and the bv add
    all happen on the host after the gather.  Each dma_start costs ~800ns
    of serialized DIRECT2D descriptor generation on its engine's sequencer,
    so the final block issues h1's store on the (by then idle) scalar ring
    to overlap the two generations.
    """
    jsl = slice(512 * j, 512 * (j + 1))
    for h in range(2):
        st = s.store_pool.tile([128, 512], BF16, tag="store", name=f"st{h}")
        nc.vector.tensor_copy(out=st[0:65, :], in_=py[h][0:65, :])
        eng = nc.scalar if (last and h == 1) else nc.sync
        eng.dma_start(out=s.out[b, h, :, jsl], in_=st[0:65, :])
        if not last:
            yield


def _pv_mms(nc, py, v_sb, ex, i):
    for h in range(2):
        nc.tensor.matmul(
            py[h][:, :], v_sb[:, i, 64 * h:64 * h + 128],
            ex[:, 512 * h:512 * (h + 1)],
            start=(i == 0), stop=(i == SK - 1))


def _emit_body(nc, tc, ctx, aps):
    from collections import deque

    s = _setup(nc, tc, ctx, aps)
    # Tile derives dependencies from emission order, so every projection
    # block must be emitted before the attention matmul that reads it.
    # Projection chunks (~2 matmuls each) are interleaved into the previous
    # batch's attention emission so the TensorEngine stays fed while the
    # (serial) softmax Exp chain runs.  Before attention(b) starts, proj(b)
    # must have reached its "KV" marker (all of kT/v plus qT block 0); the
    # trailing q blocks spill into attention(b)'s early i-steps, which are
    # otherwise PE-light (the PV software pipeline hasn't filled yet).
    pending = deque()
    kv_seen = {}
    gp = {b: _gen_proj(nc, s, b, split=(b == 0)) for b in range(B)}

    def drain_one():
        if not pending:
            return
        tok = next(pending[0], None)
        if tok is None:
            pending.popleft()
        elif tok == "KV":
            kv_seen[pending[0]] = True

    pending.append(gp[0])
    for tok in _gen_attn_all(nc, s):
        if tok[0] == "batch":
            b = tok[1]
            # proj(b) must be emitted through its KV marker (all kT/v plus
            # qT block 0) before this batch's first scores matmul
            while pending and not kv_seen.get(gp[b]):
                drain_one()
            if b + 1 < B:
                pending.append(gp[b + 1])
        else:
            # batch-0 leftovers at 4x so attention(0) never outruns the q
            # blocks it reads (emission order defines deps)
            n = 4 if (pending and pending[0] is gp[0]) else 1
            for _ in range(n):
                drain_one()
    while pending:
        drain_one()


def _build():
    from contextlib import ExitStack

    nc = bacc.Bacc("TRN2", target_bir_lowering=False, debug=False)
    x = nc.dram_tensor("x", [B, 8, 128, S], BF16, kind="ExternalInput").ap()
    wq = nc.dram_tensor("wcat", [128, 3, 8, 128], BF16,
                        kind="ExternalInput").ap()
    bq = nc.dram_tensor("consts", [128, 66], F32, kind="ExternalInput").ap()
    out = nc.dram_tensor("out", [B, 2, 65, S], BF16, kind="ExternalOutput").ap()
    aps = (x, wq, bq, out)
    with tile.TileContext(nc) as tc:
        with ExitStack() as ctx:
            _emit_body(nc, tc, ctx, aps)
    nc.compile()
    return nc


_BUILD_CACHE = {}


def _get_built():
    if "nc" not in _BUILD_CACHE:
        _BUILD_CACHE["nc"] = _build()
    return _BUILD_CACHE["nc"]


def kernel(x, mask, Wq, bq, Wk, bk, Wv, bv):
    global LAST_RESULTS
    bf16 = ml_dtypes.bfloat16
    x_bf = np.asarray(x, dtype=np.float32).astype(bf16)
    # [B, S, E] -> [B, 8, 128, S]: E-chunk-transposed on the host so every
    # device-side DMA is a plain linear load.
    x_t = np.ascontiguousarray(
        x_bf.reshape(B, S, 8, 128).transpose(0, 2, 3, 1))
    mask_f = np.asarray(mask).astype(np.float32)
    maskb = (mask_f - 1.0) * (-NEG)  # 0 where mask==1, NEG where mask==0
    maskb = np.ascontiguousarray(
        maskb.reshape(B, S // 128, 128).transpose(2, 0, 1)).astype(np.float32)

    nc = _get_built()

    in_maps = []
    for c in range(NCORES):
        sl = slice(DHC * c, DHC * (c + 1))

        def warr(w):
            w = np.asarray(w, dtype=np.float32)[:, sl].astype(bf16)
            return np.ascontiguousarray(
                w.reshape(8, 128, 128).transpose(1, 0, 2))

        wcat = np.stack([warr(Wq), warr(Wk), warr(Wv)], axis=1)
        consts = np.empty((128, 66), dtype=np.float32)
        consts[:, 0] = np.asarray(bq, dtype=np.float32)[sl] / 8.0
        consts[:, 1] = np.asarray(bk, dtype=np.float32)[sl]
        consts[:, 2:66] = maskb.reshape(128, 64)
        in_maps.append({
            "x": x_t,
            "wcat": np.ascontiguousarray(wcat),
            "consts": consts,
        })

    res = run_bass_kernel_spmd(nc, in_maps, core_ids=list(range(NCORES)))
    LAST_RESULTS = res

    # Host-side unshard: normalize by the softmax denominator row, put the
    # heads back on the feature axis, and add bv (exact: softmax weights
    # sum to 1, so y = sum(w * (v + bv)) == sum(w * v) + bv).
    parts = []
    for c in range(NCORES):
        r = np.asarray(res.results[c]["out"], dtype=np.float32)  # [B,2,65,S]
        # v_aug packing: head0 rows 0:64 are y with denom in row 64;
        # head1 row 0 is denom with y in rows 1:65.
        y0 = r[:, 0, 0:64, :] / r[:, 0, 64:65, :]
        y1 = r[:, 1, 1:65, :] / r[:, 1, 0:1, :]
        y = np.stack([y0, y1], axis=1)  # [B, 2, 64, S]
        parts.append(y.transpose(0, 3, 1, 2).reshape(B, S, DHC))
    full = np.concatenate(parts, axis=-1)
    bvf = np.asarray(bv, dtype=np.float32)
    if np.any(bvf != 0):
        full = full + bvf[None, None, :]
    return np.ascontiguousarray(full, dtype=np.float32)

